# revision 1
# baseline (speedup 1.0000x reference)
"""SSN superpixel forward (ASTSFormer) on 8 Trainium2 cores.

Reference semantics: input (2,3,1024,1024); output (1,3,1024,1024) depends only
on batch 0.  STOKEN=16 -> 64x64 grid of S=4096 cells; per pixel the 9 candidate
superpixels are flat-offset neighbors {-65,-64,-63,-1,0,1,63,64,65} of its home
cell (flat wraparound at row ends, valid iff 0<=cand<S).  N_ITER=2 -> one soft
centroid update, then a final affinity pass whose argmax gives hard labels,
then segment-mean recoloring.

Kernel strategy (per core k of 8, owning image rows [128k, 128k+128)):
 - dense stencil: all pixels of a cell share the 9 candidate cells, so the
   candidate-centroid "gather" is a broadcast of a 768-cell window
   [512k-128, 512k+640) (12 cell rows; 16x expanded along columns), and the
   per-pixel weight "scatter" is a per-cell reduction accumulated into
   3x3-shifted positions of the same window.
 - out-of-range cells carry a BIG sentinel centroid so their affinity
   underflows to exactly 0 (reproducing the reference's valid-mask).
 - e_k = 2<x,g_k> - |g_k|^2 replaces the squared distance (|x|^2 cancels in
   softmax; argmax unchanged).
 - per-cell sums via segmented free-dim reduce + one-hot matmuls on the PE
   (row shift dr encoded in the stationary one-hot; flat column wrap handled
   by a 4-wide stripe matmul with the dr+-1 one-hot; quantity index q
   interleaved as col = 4*jc + q so shifted accumulation stays contiguous).
 - core-dependent window placement on the global 64x64 grid is done by
   per-core one-hot placement matrices on the PE (no dynamic addressing);
   the two global reductions are 64KB AllReduces.
"""
import sys

sys.path.insert(0, "/opt/trn_rl_repo")

import numpy as np
import bass_rust
from concourse import bass, bacc, mybir, tile

F32 = mybir.dt.float32
Alu = mybir.AluOpType
Act = mybir.ActivationFunctionType

BIG = 1e18
N_CORES = 8
H = W = 1024
C = 3
S = 4096


def mkap(ap, offset, dims):
    c = ap.copy()
    c.offset = offset
    c.ap = bass_rust.VecI64Pair(dims)
    return c


def mkfree(ap, extra_offset, free_dims):
    """Replace the FREE dims of an SBUF/PSUM AP, keeping its partition dim."""
    c = ap.copy()
    orig = [list(p) for p in c.ap]
    c.offset = c.offset + extra_offset
    c.ap = bass_rust.VecI64Pair([orig[0]] + free_dims)
    return c


def rep4(ap1024):
    """(128,1024) tile AP -> (128,4,1024) 4x re-read view."""
    return ap1024.unsqueeze(1).broadcast_to((128, 4, 1024))


def build_nc():
    nc = bacc.Bacc("TRN2", target_bir_lowering=False, debug=False,
                   num_devices=N_CORES)

    xs_d = nc.dram_tensor("xs", [C, 192, W], F32, kind="ExternalInput")
    whot_d = nc.dram_tensor("whot", [128, 16], F32, kind="ExternalInput")
    whalo_d = nc.dram_tensor("whalo", [64, 12], F32, kind="ExternalInput")
    maskA_d = nc.dram_tensor("maskA", [12, 192], F32, kind="ExternalInput")
    place_d = nc.dram_tensor("place", [12, 64], F32, kind="ExternalInput")
    placeT_d = nc.dram_tensor("placeT", [64, 12], F32, kind="ExternalInput")
    out_d = nc.dram_tensor("out", [C, 128, W], F32, kind="ExternalOutput")

    with tile.TileContext(nc) as tc:
        with tc.tile_pool(name="pp", bufs=1) as pp, \
             tc.tile_pool(name="wk", bufs=2) as wkp, \
             tc.tile_pool(name="ps", bufs=1, space="PSUM") as psp, \
             tc.tile_pool(name="dr", bufs=1, space="DRAM") as dp:

            # ---------------- stage A: load, block means, init centroids ----
            xt = []   # own 128 rows per channel
            xh = []   # halo rows (32 top + 32 bottom) per channel
            for c in range(C):
                t = pp.tile([128, W], F32, tag=f"x{c}", name=f"x{c}")
                nc.sync.dma_start(out=t[:, :], in_=xs_d[c, 32:160, :])
                xt.append(t)
                h = wkp.tile([64, W], F32, tag=("t", "t2", "xh2")[c],
                             name=f"xh{c}", bufs=1)
                nc.sync.dma_start(out=h[0:32, :], in_=xs_d[c, 0:32, :])
                nc.sync.dma_start(out=h[32:64, :], in_=xs_d[c, 160:192, :])
                xh.append(h)
            whot = pp.tile([128, 16], F32, tag="whot", name="whot")
            nc.sync.dma_start(out=whot[:, :], in_=whot_d[:, :])
            whalo = pp.tile([64, 12], F32, tag="whalo", name="whalo")
            nc.sync.dma_start(out=whalo[:, :], in_=whalo_d[:, :])
            maskA = pp.tile([12, 192], F32, tag="maskA", name="maskA")
            nc.sync.dma_start(out=maskA[:, :], in_=maskA_d[:, :])
            place = pp.tile([12, 64], F32, tag="place", name="place")
            nc.sync.dma_start(out=place[:, :], in_=place_d[:, :])
            placeT = pp.tile([64, 12], F32, tag="placeT", name="placeT")
            nc.sync.dma_start(out=placeT[:, :], in_=placeT_d[:, :])

            zlh = pp.tile([1, 256], F32, tag="zlh", name="zlh")
            nc.vector.memset(zlh[:, :], 0.0)

            # tiny dummy AllReduce posted early: the CC core absorbs the
            # per-core launch stagger in parallel with compute, so the first
            # real AllReduce only waits on compute skew.
            dummy_in = dp.tile([64], F32, tag="dummy_in", name="dummy_in")
            nc.sync.dma_start(out=dummy_in[:], in_=whot_d[0:4, 0:16])
            dummy_out = dp.tile([64], F32, tag="dummy_out", name="dummy_out")
            nc.gpsimd.collective_compute(
                "AllReduce", Alu.add, replica_groups=[list(range(N_CORES))],
                ins=[dummy_in[:].opt()], outs=[dummy_out[:].opt()])


            # X4 = [ones | x0 | x1 | x2]  (128, 4096)
            X4 = pp.tile([128, 4096], F32, tag="X4", name="X4")
            nc.gpsimd.memset(X4[:, 0:1024], 1.0)
            for c in range(C):
                nc.gpsimd.tensor_copy(X4[:, (1 + c) * 1024:(2 + c) * 1024], xt[c][:, :])

            # block sums -> P1 (12, 192) = per-window-cell sums per channel
            P1 = psp.tile([12, 192], F32, tag="P1", name="P1")
            nc.tensor.matmul(P1[:, :], zlh[0:1, 0:12], zlh[0:1, 0:192],
                             start=True, stop=False)
            for c in range(C):
                rs = wkp.tile([128, 64], F32, tag="rs", name="rs", bufs=1)
                nc.vector.tensor_reduce(
                    rs[:, :], xt[c][:, :].rearrange("p (c s) -> p c s", s=16),
                    axis=mybir.AxisListType.X, op=Alu.add)
                nc.tensor.matmul(P1[:, c * 64:(c + 1) * 64], whot[:, 2:14],
                                 rs[:, :], start=False, stop=False)
                rsh = wkp.tile([64, 64], F32, tag="rsh", name="rsh", bufs=1)
                nc.vector.tensor_reduce(
                    rsh[:, :], xh[c][:, :].rearrange("p (c s) -> p c s", s=16),
                    axis=mybir.AxisListType.X, op=Alu.add)
                last = c == C - 1
                nc.tensor.matmul(P1[:, c * 64:(c + 1) * 64], whalo[:, :],
                                 rsh[:, :], start=False, stop=last)

            # cw = P1/256 + maskA  (sentinel for out-of-range cells)
            cw = pp.tile([12, 192], F32, tag="cw", name="cw")
            nc.vector.scalar_tensor_tensor(cw[:, :], P1[:, :], 1.0 / 256.0,
                                           maskA[:, :], op0=Alu.mult, op1=Alu.add)
            # gg row: sum over channels of cw^2 -> (12, 64)
            sqA = pp.tile([12, 192], F32, tag="sqA", name="sqA")
            nc.vector.tensor_tensor(sqA[:, :], cw[:, :], cw[:, :], op=Alu.mult)
            ggrow = pp.tile([12, 64], F32, tag="ggrow", name="ggrow")
            nc.vector.tensor_reduce(
                ggrow[:, :], sqA[:, :].rearrange("p (c j) -> p j c", c=3),
                axis=mybir.AxisListType.X, op=Alu.add)

            # flat cent + gg to DRAM, then 16x expansion
            centflat = dp.tile([C * 768], F32, tag="centflat", name="centflat")
            for c in range(C):
                nc.sync.dma_start(out=centflat[c * 768:(c + 1) * 768],
                                  in_=cw[:, c * 64:(c + 1) * 64])
            ggflat = dp.tile([768], F32, tag="ggflat", name="ggflat")
            nc.sync.dma_start(out=ggflat[:], in_=ggrow[:, :])

            # ---------------- helpers --------------------------------------
            def g_tiles(dr, cflat, ggflat_, tagsuf):
                cellv = wkp.tile([128, 264], F32, tag="cellv", name=f"cellv{tagsuf}", bufs=1)
                for c in range(C):
                    nc.sync.dma_start(out=cellv[:, c * 66:(c + 1) * 66], in_=mkap(
                        cflat[:], c * 768 + (2 + dr) * 64 - 1,
                        [[64, 8], [0, 16], [1, 66]]))
                nc.sync.dma_start(out=cellv[:, 198:264], in_=mkap(
                    ggflat_[:], (2 + dr) * 64 - 1, [[64, 8], [0, 16], [1, 66]]))
                gB = wkp.tile([128, 4224], F32, tag="gB", name=f"gB{tagsuf}")
                nc.scalar.copy(
                    gB[:, :].rearrange("p (b j u) -> p b j u", b=4, u=16),
                    mkfree(cellv[:, :], 0, [[66, 4], [1, 66], [0, 16]]))
                gs = [gB[:, c * 1056:(c + 1) * 1056] for c in range(C)]
                return gs, gB[:, 3 * 1056:4224]

            def rep3(x1024):
                return mkfree(x1024, 0, [[0, 3], [1, 1024]])

            def dc3(gview):
                # three dc-shifted 1024-slices of a (128,1056) view as (128,3,1024)
                return mkfree(gview, 0, [[16, 3], [1, 1024]])

            def compute_e3(e3, gs, gg):
                """e3 (128, 3072): e for dc=-1,0,1 in 1024-blocks, one dr."""
                t = wkp.tile([128, 3072], F32, tag="t", name="t", bufs=1)
                t2 = wkp.tile([128, 3072], F32, tag="t2", name="t2", bufs=1)
                v3 = lambda a: a[:, :].rearrange("p (d b) -> p d b", d=3)
                nc.vector.tensor_tensor(v3(t), rep3(xt[0][:, :]), dc3(gs[0]),
                                        op=Alu.mult)
                nc.vector.tensor_tensor(v3(t2), rep3(xt[1][:, :]), dc3(gs[1]),
                                        op=Alu.mult)
                nc.vector.tensor_tensor(t[:, :], t[:, :], t2[:, :], op=Alu.add)
                nc.vector.tensor_tensor(v3(t2), rep3(xt[2][:, :]), dc3(gs[2]),
                                        op=Alu.mult)
                nc.vector.tensor_tensor(t[:, :], t[:, :], t2[:, :], op=Alu.add)
                nc.vector.scalar_tensor_tensor(
                    e3[:, :].rearrange("p (d b) -> p d b", d=3), v3(t), 2.0,
                    dc3(gg), op0=Alu.mult, op1=Alu.subtract)

            def scatter_cellsums(Pacc, src4, dr, dc, first, last):
                """src4 (128, 4096) quantity-stacked products; segmented-reduce to
                (128, 256) interleaved as col=jc*4+q, then accumulate into Pacc
                window rows (shift dr via one-hot lhsT, dc via contiguous col
                shift, flat wrap via a 4-wide stripe with the dr+-1 one-hot)."""
                rq = wkp.tile([128, 256], F32, tag="rq", name="rq")
                nc.vector.tensor_reduce(
                    rq[:, :].rearrange("p (c a) -> p a c", a=4),
                    src4[:, :].rearrange("p (a c s) -> p a c s", a=4, s=16),
                    axis=mybir.AxisListType.X, op=Alu.add)
                lh = whot[:, 2 - dr:14 - dr]
                if first:
                    nc.tensor.matmul(Pacc[:, :], zlh[0:1, 0:12], zlh[0:1, 0:256],
                                     start=True, stop=False)
                if dc == 0:
                    nc.tensor.matmul(Pacc[:, :], lh, rq[:, :], start=False, stop=last)
                elif dc == 1:
                    nc.tensor.matmul(Pacc[:, 4:256], lh, rq[:, 0:252],
                                     start=False, stop=False)
                    lh2 = whot[:, 2 - (dr + 1):14 - (dr + 1)]
                    nc.tensor.matmul(Pacc[:, 0:4], lh2, rq[:, 252:256],
                                     start=False, stop=last)
                else:
                    nc.tensor.matmul(Pacc[:, 0:252], lh, rq[:, 4:256],
                                     start=False, stop=False)
                    lh2 = whot[:, 2 - (dr - 1):14 - (dr - 1)]
                    nc.tensor.matmul(Pacc[:, 252:256], lh2, rq[:, 0:4],
                                     start=False, stop=last)

            def ar_launch(Pacc, tagsuf):
                """(12,256) window partials -> place on global (64,256) grid ->
                launch AllReduce; returns the AR output DRAM tile."""
                a2s = pp.tile([12, 256], F32, tag="a2s", name=f"a2s{tagsuf}")
                nc.scalar.copy(a2s[:, :], Pacc[:, :])
                PG = psp.tile([64, 256], F32, tag="PG", name=f"PG{tagsuf}")
                nc.tensor.matmul(PG[:, :], place[:, :], a2s[:, :],
                                 start=True, stop=True)
                pgs = pp.tile([64, 256], F32, tag="pgs", name=f"pgs{tagsuf}")
                nc.scalar.copy(pgs[:, :], PG[:, :])
                ar_in = dp.tile([16384], F32, tag=f"ar{tagsuf}_in",
                                name=f"ar{tagsuf}_in")
                nc.sync.dma_start(out=ar_in[:], in_=pgs[:, :])
                ar_out = dp.tile([16384], F32, tag=f"ar{tagsuf}_out",
                                 name=f"ar{tagsuf}_out")
                nc.gpsimd.collective_compute(
                    "AllReduce", Alu.add, replica_groups=[list(range(N_CORES))],
                    ins=[ar_in[:].opt()], outs=[ar_out[:].opt()])
                return ar_out

            def global_reduce(Pacc, tagsuf):
                ar_out = ar_launch(Pacc, tagsuf)
                ars = pp.tile([64, 256], F32, tag="pgs", name=f"ars{tagsuf}")
                nc.sync.dma_start(out=ars[:, :], in_=ar_out[:])
                CMB = psp.tile([12, 256], F32, tag="CMB", name=f"CMB{tagsuf}")
                nc.tensor.matmul(CMB[:, :], placeT[:, :], ars[:, :],
                                 start=True, stop=True)
                return CMB

            # ---------------- pass 1: soft affinity + weighted cell sums ---
            W3 = [pp.tile([128, 3072], F32, tag=f"W3{d}", name=f"W3{d}")
                  for d in range(3)]
            wtiles = [W3[(k // 3)][:, (k % 3) * 1024:(k % 3 + 1) * 1024]
                      for k in range(9)]
            Z = pp.tile([128, 1024], F32, tag="Z", name="Z")
            for dr in (-1, 0, 1):
                gs, gg = g_tiles(dr, centflat, ggflat, "a")
                et = wkp.tile([128, 3072], F32, tag="prod", name="et", bufs=2)
                compute_e3(et, gs, gg)
                nc.scalar.activation(W3[dr + 1][:, :], et[:, :], Act.Exp)
                # Z += per-pixel sum over the 3 dc blocks (innermost-d reduce)
                zpart = wkp.tile([128, 1024], F32, tag="zpart", name="zpart")
                nc.vector.tensor_reduce(
                    zpart[:, :],
                    mkfree(W3[dr + 1][:, :], 0, [[1, 1024], [1024, 3]]),
                    axis=mybir.AxisListType.X, op=Alu.add)
                if dr == -1:
                    nc.vector.tensor_copy(Z[:, :], zpart[:, :])
                else:
                    nc.vector.tensor_tensor(Z[:, :], Z[:, :], zpart[:, :],
                                            op=Alu.add)
            rinv = pp.tile([128, 1024], F32, tag="rinv", name="rinv")
            nc.vector.reciprocal(rinv[:, :], Z[:, :])
            Y4 = pp.tile([128, 4096], F32, tag="Y4", name="Y4")
            nc.vector.tensor_tensor(
                Y4[:, :].rearrange("p (a b) -> p a b", a=4),
                X4[:, :].rearrange("p (a b) -> p a b", a=4),
                rep4(rinv[:, :]), op=Alu.mult)
            P2a = psp.tile([12, 256], F32, tag="P2a", name="P2a")
            P2b = psp.tile([12, 256], F32, tag="P2b", name="P2b")
            halves = []
            for dr in (-1, 0, 1):
                for dc in (-1, 0, 1):
                    k = (dr + 1) * 3 + (dc + 1)
                    P2, first, last = (P2a, k == 0, k == 3) if k <= 3 else \
                                      (P2b, k == 4, k == 8)
                    eng1 = nc.vector
                    prod = wkp.tile([128, 4096], F32, tag="prod", name="prod", bufs=2)
                    eng1.tensor_tensor(
                        prod[:, :].rearrange("p (a b) -> p a b", a=4),
                        Y4[:, :].rearrange("p (a b) -> p a b", a=4),
                        rep4(wtiles[k][:, :]), op=Alu.mult)
                    scatter_cellsums(P2, prod, dr, dc, first=first, last=last)
                    if last:
                        halves.append(ar_launch(P2, "1a" if P2 is P2a else "1b"))

            # ---------------- AllReduce 1 -> spf ---------------------------
            CMB1 = psp.tile([12, 256], F32, tag="CMB", name="CMB1")
            for i, ar_out in enumerate(halves):
                ars = pp.tile([64, 256], F32, tag="pgs", name=f"ars1{i}")
                nc.sync.dma_start(out=ars[:, :], in_=ar_out[:])
                nc.tensor.matmul(CMB1[:, :], placeT[:, :], ars[:, :],
                                 start=(i == 0), stop=(i == 1))
            denp = pp.tile([12, 64], F32, tag="denp", name="denp")
            nc.vector.tensor_scalar_add(
                denp[:, :], mkfree(CMB1[:, :], 0, [[4, 64]]), 1e-16)
            rinvD = pp.tile([12, 64], F32, tag="rinvD", name="rinvD")
            nc.vector.reciprocal(rinvD[:, :], denp[:, :])
            spf = pp.tile([12, 192], F32, tag="spf", name="spf")
            nc.vector.tensor_tensor(
                spf[:, :].rearrange("p (c j) -> p c j", c=3),
                mkfree(CMB1[:, :], 1, [[1, 3], [4, 64]]),
                rinvD[:, :].unsqueeze(1).broadcast_to((12, 3, 64)), op=Alu.mult)
            nc.vector.tensor_tensor(spf[:, :], spf[:, :], maskA[:, :], op=Alu.add)
            # gg2 = sum over channels of spf^2
            sq2 = pp.tile([12, 192], F32, tag="sqA", name="sq2")
            nc.vector.tensor_tensor(sq2[:, :], spf[:, :], spf[:, :], op=Alu.mult)
            gg2row = pp.tile([12, 64], F32, tag="ggrow", name="gg2row")
            nc.vector.tensor_reduce(
                gg2row[:, :], sq2[:, :].rearrange("p (c j) -> p j c", c=3),
                axis=mybir.AxisListType.X, op=Alu.add)
            centflat2 = dp.tile([C * 768], F32, tag="centflat2", name="centflat2")
            for c in range(C):
                nc.sync.dma_start(out=centflat2[c * 768:(c + 1) * 768],
                                  in_=spf[:, c * 64:(c + 1) * 64])
            ggflat2 = dp.tile([768], F32, tag="ggflat2", name="ggflat2")
            nc.sync.dma_start(out=ggflat2[:], in_=gg2row[:, :])

            # ---------------- pass 2: final affinity, argmax, hard sums ----
            for dr in (-1, 0, 1):
                gs, gg = g_tiles(dr, centflat2, ggflat2, "b")
                compute_e3(W3[dr + 1], gs, gg)
            best = pp.tile([128, 1024], F32, tag="Z", name="best")
            bpart = [wkp.tile([128, 1024], F32, tag="zpart", name=f"bp{d}")
                     for d in range(3)]
            for d in range(3):
                nc.vector.tensor_reduce(
                    bpart[d][:, :],
                    mkfree(W3[d][:, :], 0, [[1, 1024], [1024, 3]]),
                    axis=mybir.AxisListType.X, op=Alu.max)
            nc.vector.tensor_tensor(best[:, :], bpart[0][:, :], bpart[1][:, :],
                                    op=Alu.max)
            nc.vector.tensor_tensor(best[:, :], best[:, :], bpart[2][:, :],
                                    op=Alu.max)
            for d in range(3):
                nc.vector.tensor_tensor(
                    W3[d][:, :].rearrange("p (d b) -> p d b", d=3),
                    W3[d][:, :].rearrange("p (d b) -> p d b", d=3),
                    mkfree(best[:, :], 0, [[0, 3], [1, 1024]]), op=Alu.is_equal)
            P3a = psp.tile([12, 256], F32, tag="P2a", name="P3a")
            P3b = psp.tile([12, 256], F32, tag="P2b", name="P3b")
            halves2 = []
            for dr in (-1, 0, 1):
                for dc in (-1, 0, 1):
                    k = (dr + 1) * 3 + (dc + 1)
                    P3, first, last = (P3a, k == 0, k == 3) if k <= 3 else \
                                      (P3b, k == 4, k == 8)
                    eng = nc.gpsimd
                    prod = wkp.tile([128, 4096], F32, tag="prod", name="prod", bufs=2)
                    eng.tensor_tensor(
                        prod[:, :].rearrange("p (a b) -> p a b", a=4),
                        X4[:, :].rearrange("p (a b) -> p a b", a=4),
                        rep4(wtiles[k][:, :]), op=Alu.mult)
                    scatter_cellsums(P3, prod, dr, dc, first=first, last=last)
                    if last:
                        halves2.append(ar_launch(P3, "2a" if P3 is P3a else "2b"))

            # ---------------- AllReduce 2 -> means -------------------------
            CMB2 = psp.tile([12, 256], F32, tag="CMB", name="CMB2")
            for i, ar_out in enumerate(halves2):
                ars = pp.tile([64, 256], F32, tag="pgs", name=f"ars2{i}")
                nc.sync.dma_start(out=ars[:, :], in_=ar_out[:])
                nc.tensor.matmul(CMB2[:, :], placeT[:, :], ars[:, :],
                                 start=(i == 0), stop=(i == 1))
            mcnt = pp.tile([12, 64], F32, tag="denp", name="mcnt")
            nc.vector.tensor_scalar_max(
                mcnt[:, :], mkfree(CMB2[:, :], 0, [[4, 64]]), 1.0)
            rinvM = pp.tile([12, 64], F32, tag="rinvD", name="rinvM")
            nc.vector.reciprocal(rinvM[:, :], mcnt[:, :])
            means = pp.tile([12, 192], F32, tag="spf", name="means")
            nc.vector.tensor_tensor(
                means[:, :].rearrange("p (c j) -> p c j", c=3),
                mkfree(CMB2[:, :], 1, [[1, 3], [4, 64]]),
                rinvM[:, :].unsqueeze(1).broadcast_to((12, 3, 64)), op=Alu.mult)
            meansflat = dp.tile([C * 768], F32, tag="meansflat", name="meansflat")
            for c in range(C):
                nc.sync.dma_start(out=meansflat[c * 768:(c + 1) * 768],
                                  in_=means[:, c * 64:(c + 1) * 64])

            # ---------------- recolor: select means by argmax mask ---------
            ot = [pp.tile([128, 1024], F32, tag="rinv", name="o0"),
                  wkp.tile([128, 1024], F32, tag="t", name="o1", bufs=1),
                  wkp.tile([128, 1024], F32, tag="t2", name="o2", bufs=1)]
            for c in range(C):
                nc.gpsimd.memset(ot[c][:, :], 0.0)
            for dr in (-1, 0, 1):
                cellm = wkp.tile([128, 264], F32, tag="cellv", name="cellm", bufs=1)
                for c in range(C):
                    nc.sync.dma_start(out=cellm[:, c * 66:(c + 1) * 66], in_=mkap(
                        meansflat[:], c * 768 + (2 + dr) * 64 - 1,
                        [[64, 8], [0, 16], [1, 66]]))
                gmB = wkp.tile([128, 4224], F32, tag="gB", name="gmB")
                nc.scalar.copy(
                    gmB[:, 0:3168].rearrange("p (b j u) -> p b j u", b=3, u=16),
                    mkfree(cellm[:, :], 0, [[66, 3], [1, 66], [0, 16]]))
                gms = [gmB[:, c * 1056:(c + 1) * 1056] for c in range(C)]
                for dc in (-1, 0, 1):
                    k = (dr + 1) * 3 + (dc + 1)
                    lo = 16 * (dc + 1)
                    for c in range(C):
                        nc.vector.select(ot[c][:, :],
                                         wtiles[k][:, :].bitcast(mybir.dt.int32),
                                         gms[c][:, lo:lo + 1024], ot[c][:, :])
            for c in range(C):
                nc.sync.dma_start(out=out_d[c, :, :], in_=ot[c][:, :])

    nc.compile()
    return nc


# ---------------- host side ------------------------------------------------

def make_inputs(pixel_features):
    """pixel_features (2,3,1024,1024) -> list of 8 per-core input dicts."""
    img = np.ascontiguousarray(np.asarray(pixel_features, np.float32)[0])  # (3,H,W)
    whot = np.zeros((128, 16), np.float32)
    for q in range(8):
        whot[q * 16:(q + 1) * 16, q + 4] = 1.0
    whalo = np.zeros((64, 12), np.float32)
    for i, m in enumerate((0, 1, 10, 11)):
        whalo[i * 16:(i + 1) * 16, m] = 1.0
    in_maps = []
    for k in range(N_CORES):
        lo, hi = 128 * k - 32, 128 * k + 160
        slab = np.zeros((C, 192, W), np.float32)
        glo, ghi = max(lo, 0), min(hi, H)
        slab[:, glo - lo:ghi - lo, :] = img[:, glo:ghi, :]
        cells = np.arange(512 * k - 128, 512 * k + 640)
        oob = (cells < 0) | (cells >= S)
        maskA = np.where(oob, np.float32(BIG), np.float32(0.0)).reshape(12, 64)
        maskA = np.concatenate([maskA] * 3, axis=1)  # (12, 192) ch-blocks
        place = np.zeros((12, 64), np.float32)
        for r in range(12):
            m = 8 * k - 2 + r
            if 0 <= m < 64:
                place[r, m] = 1.0
        in_maps.append({
            "xs": slab, "whot": whot, "whalo": whalo,
            "maskA": np.ascontiguousarray(maskA),
            "place": place, "placeT": np.ascontiguousarray(place.T),
        })
    return in_maps


_NC_CACHE = None


def get_nc():
    global _NC_CACHE
    if _NC_CACHE is None:
        _NC_CACHE = build_nc()
    return _NC_CACHE


def kernel(pixel_features):
    from concourse.bass_utils import run_bass_kernel_spmd
    nc = get_nc()
    in_maps = make_inputs(pixel_features)
    res = run_bass_kernel_spmd(nc, in_maps, core_ids=list(range(N_CORES)))
    out = np.empty((1, C, H, W), np.float32)
    for k in range(N_CORES):
        out[0, :, 128 * k:128 * (k + 1), :] = \
            np.asarray(res.results[k]["out"]).reshape(C, 128, W)
    return out

